# revision 22
# baseline (speedup 1.0000x reference)
"""MultiHeadAttention (B=2, S=2048, HID=1024, NH=16, HD=64, RoPE) on 8 TRN2 cores.

Sharding: 8 cores = 2 batches x 4 head-groups (4 heads per core).
Per core: q/k/v projections for its 4 heads (tensor parallel on H), RoPE,
attention, and a partial o-projection over its 256 channels. Host sums the
4 partial o-projections per batch (the TP unshard) and adds bo.

All compute tensors are bf16 (x, weights, RoPE tables, q/k/v, softmax
weights); matmuls accumulate in f32 PSUM, so the end-to-end error stays
<1e-2 of the output max (fp8 attention breaches the 2e-2 gate on the q.k
path alone).

RoPE without cross-partition ops: the q/k projection weight columns are
split into an L set (channels 0-31 of each head) and an H set (channels
32-63), so each PSUM partition holds a channel and its rotate-half partner
at the same partition index in two PSUM banks. RoPE is then two full-width
DVE muls (the sin term reads the PSUM pair dim reversed; signs live in the
sin table) plus one Pool add -> bf16 [128, 2, S]. DMAs then reshuffle
partitions into the [channel, seq] layout the scores matmul needs (DMA
moves across partitions for free; engines cannot).

Attention: scores per (head, k-tile) are bf16 matmuls ([64,128] x
[64,512]); exp on ACT with scale=1/8, bias=-4 (fits the score outliers;
the bias cancels exactly in the row-sum normalization; ACT exp is the
critical engine at ~133us, so everything else hides under it). AV runs
transposed -- p [128k,128q] stationary, v [128k,65] moving (65th col =
ones accumulates the row sums) -- packing the 64-wide head dim into the
free axis at full 128-partition occupancy, which halves AV PE time.
Normalization is a per-partition tensor_scalar mul with the reciprocal row
sums; DMA transposes ([128,128] bf16 via the SBUF crossbar) put the
normalized attention back in [channel, seq] for the o-projection.

Scheduling: the Tile scheduler only orders a consumer against producers
already emitted, so every consumer is emitted after its producers. The
HWDGE queue serializes at ~625ns/DMA, so input DMAs are merged to one per
tensor/chunk and secondary traffic (late q reshuffles, output stores) goes
through the software DGE on the idle Pool engine. exp starts ~18us in; v
tiles and late q chunks interleave into the early attention steps under
the ACT budget, with the AV pipeline lagging scores by 3 steps so all v
producers are emitted before the first AV.

PSUM (8 banks): ring for qk-proj chunks + score tiles [128,2,512]x2 (4) +
v-proj [128,256] (1) + transposed-AV accumulator [128,4,65] (1) + o-proj
ring [128,512]x2 (2).
"""

import numpy as np

B, S, HID = 2, 2048, 1024
NH, HD = 16, 64
BASE = 10000.0
N_CORES = 8
GROUPS = 4                 # head groups (tensor parallel)
HPC = NH // GROUPS         # heads per core = 4
CPC = HPC * HD             # channels per core = 256
SC = 512                   # seq chunk (matmul free dim)
NSC = S // SC              # 4
NST = S // 128             # 16 s-tiles / k-tiles
KO = HID // 128            # 8 contraction slices for projections
VW = HD + 1                # v row stride per head (64 + ones col)

_cached = None


def _split_waits(nc, mybir, limit=1):
    """This walrus build accepts at most one embedded sync wait per
    instruction; hoist the rest onto NoOps just before it on the same engine."""
    n = 0
    for f in nc.m.functions:
        for b in f.blocks:
            out = []
            changed = False
            for inst in b.instructions:
                si = inst.sync_info
                waits = list(si.on_wait) if (si and si.on_wait) else []
                if len(waits) > limit:
                    keep = waits[-limit:]
                    excess = waits[:-limit]
                    for ci in range(0, len(excess), limit):
                        out.append(mybir.InstNoOp(
                            name=f"{inst.name}-wsplit-{ci}",
                            engine=inst.engine,
                            sync_info=mybir.SyncInfo(
                                on_wait=excess[ci:ci + limit], on_update=[]),
                            bass_nofuse=True,
                        ))
                        n += 1
                    inst.sync_info = mybir.SyncInfo(
                        on_wait=keep,
                        on_update=(list(si.on_update) if si else []))
                    changed = True
                out.append(inst)
            if changed:
                b.instructions = out
    return n


def _build():
    import concourse.bass as bass
    import concourse.mybir as mybir
    import concourse.tile as tile

    f32 = mybir.dt.float32
    bf16 = mybir.dt.bfloat16
    AF = mybir.ActivationFunctionType

    nc = bass.Bass()
    xT = nc.dram_tensor("xT", [HID, S], bf16, kind="ExternalInput")
    wqT = nc.dram_tensor("wqT", [HID, CPC], bf16, kind="ExternalInput")
    wkT = nc.dram_tensor("wkT", [HID, CPC], bf16, kind="ExternalInput")
    wvT = nc.dram_tensor("wvT", [HID, CPC], bf16, kind="ExternalInput")
    woT = nc.dram_tensor("woT", [CPC, HID], bf16, kind="ExternalInput")
    cos2 = nc.dram_tensor("cos2", [128, 2, S], bf16, kind="ExternalInput")
    sin2 = nc.dram_tensor("sin2", [128, 2, S], bf16, kind="ExternalInput")
    out = nc.dram_tensor("out", [S, HID], f32, kind="ExternalOutput")

    with tile.TileContext(nc) as tc:
        with (
            tc.tile_pool(name="persist", bufs=1) as persist,
            tc.tile_pool(name="pb", bufs=4) as pb,
            tc.tile_pool(name="rope", bufs=2) as rope,
            tc.tile_pool(name="avq", bufs=2) as avq_pool,
            tc.tile_pool(name="ptmp", bufs=2) as ptmp,
            tc.tile_pool(name="pc", bufs=2) as pc,
            tc.tile_pool(name="xw", bufs=1) as xw,
        ):
            # ---- persistent SBUF ----
            cos_sb = persist.tile([128, 2, S], bf16)
            sin_sb = persist.tile([128, 2, S], bf16)
            wo_sb = persist.tile([128, 2, HID], bf16)
            # [c, s] layout: tile 0 = heads 0,1 (64 rows each); tile 1 = 2,3
            k_cs = [persist.tile([128, S], bf16, name=f"kcs{i}")
                    for i in range(2)]
            q_cs = [persist.tile([128, S], bf16, name=f"qcs{i}")
                    for i in range(2)]
            v_bf = [persist.tile([128, HPC * VW], bf16, name=f"vbf{t}")
                    for t in range(NST)]
            avt_sb = persist.tile([128, 2, S], bf16)
            bias_sb = persist.tile([128, 1], f32)
            nc.vector.memset(bias_sb[:], -4.0)
            onesv_f = persist.tile([128, HPC], f32)
            nc.vector.memset(onesv_f[:], 1.0)
            for t in range(NST):
                vt_ones = v_bf[t][:].rearrange("p (h w) -> p h w", w=VW)
                nc.vector.tensor_copy(out=vt_ones[:, :, HD], in_=onesv_f[:])

            x_sb = [xw.tile([128, KO, SC], bf16, name=f"x{c}")
                    for c in range(NSC)]
            wk_sb = xw.tile([128, KO, CPC], bf16, name="wk")
            wq_sb = xw.tile([128, KO, CPC], bf16, name="wq")
            wv_sb = xw.tile([128, KO, CPC], bf16, name="wv")

            def dma_w(w_sb, wdram):
                # one DMA: the DRAM side is a flat affine pattern
                nc.sync.dma_start(
                    w_sb[:], wdram[:].rearrange("(o p) c -> p o c", p=128))

            def dma_x(c):
                nc.sync.dma_start(
                    x_sb[c][:],
                    xT[:, c * SC:(c + 1) * SC].rearrange(
                        "(o p) s -> p o s", p=128))

            # DMA priority order: k path first so exp can start early.
            dma_w(wk_sb, wkT)
            dma_x(0)
            dma_x(1)
            nc.sync.dma_start(cos_sb[:], cos2[:])
            nc.sync.dma_start(sin_sb[:], sin2[:])
            dma_w(wq_sb, wqT)
            dma_x(2)
            dma_x(3)
            dma_w(wv_sb, wvT)
            for cs in range(2):
                nc.sync.dma_start(wo_sb[:, cs], woT[cs * 128:(cs + 1) * 128, :])

            # ---- projections + RoPE ----
            def qk_chunk(w_sb, dst_cs, c, pool):
                ps = pool.tile([128, 2, SC], f32, tag="mm")
                for half in range(2):           # 0 = L set, 1 = H set
                    for ko in range(KO):
                        nc.tensor.matmul(
                            ps[:, half],
                            w_sb[:, ko, half * 128:(half + 1) * 128],
                            x_sb[c][:, ko, :],
                            start=(ko == 0), stop=(ko == KO - 1),
                        )
                sl = slice(c * SC, (c + 1) * SC)
                tmc = rope.tile([128, 2, SC], bf16, tag="tmc")
                tms = rope.tile([128, 2, SC], bf16, tag="tms")
                nc.vector.tensor_mul(out=tmc[:], in0=ps[:], in1=cos_sb[:, :, sl])
                nc.vector.tensor_mul(
                    out=tms[:], in0=ps[:, ::-1], in1=sin_sb[:, :, sl])
                # add + partition reshuffle in one: out block (t, hh, d) of
                # the [c, s] layout takes LH partitions 64t+32hh at pair d
                # (DVE output may sit at a different partition base)
                for t in range(2):
                    for hh in range(2):
                        sp = 64 * t + 32 * hh
                        for d in range(2):
                            dp = 64 * hh + 32 * d
                            eng = nc.vector if d == 0 else nc.gpsimd
                            eng.tensor_add(
                                out=dst_cs[t][dp:dp + 32, sl],
                                in0=tmc[sp:sp + 32, d, :],
                                in1=tms[sp:sp + 32, d, :])

            def v_tile(st):
                ps = pv_pool.tile([128, CPC], f32, tag="pv")
                for ko in range(KO):
                    nc.tensor.matmul(
                        ps[:],
                        x_sb[st // 4][:, ko, (st % 4) * 128:(st % 4 + 1) * 128],
                        wv_sb[:, ko, :],
                        start=(ko == 0), stop=(ko == KO - 1),
                    )
                psv = ps[:].rearrange("p (h e) -> p h e", e=HD)
                vt_v = v_bf[st][:].rearrange("p (h w) -> p h w", w=VW)
                nc.vector.tensor_copy(out=vt_v[:, :, 0:HD], in_=psv[:])

            # ---- attention steps ----
            def scores_exp(qc, h):
                cs, pof = h // 2, (h % 2) * HD
                p_bf = pb.tile([128, NST, SC], bf16, tag="p_bf")
                for g in range(NST // 2):
                    sps = mm_pool.tile([128, 2, SC], f32, tag="mm")
                    for kti in range(2):
                        kt = g * 2 + kti
                        nc.tensor.matmul(
                            sps[:, kti],
                            k_cs[cs][pof:pof + HD, kt * 128:(kt + 1) * 128],
                            q_cs[cs][pof:pof + HD, qc * SC:(qc + 1) * SC],
                            start=True, stop=True,
                        )
                    nc.scalar.activation(
                        out=p_bf[:, g * 2:(g + 1) * 2], in_=sps[:],
                        func=AF.Exp, scale=0.125, bias=bias_sb[:],
                    )
                return p_bf

            def av_norm(qc, h, p_bf, av_q):
                hh = h % 2
                avb = pop_pool.tile([128, SC], f32, tag="bp", name="avb")
                avp = avb[:].rearrange("p (a b) -> p a b", b=128)
                for qt in range(4):
                    for kt in range(NST):
                        nc.tensor.matmul(
                            avp[:, qt, 0:VW],
                            p_bf[:, kt, qt * 128:(qt + 1) * 128],
                            v_bf[kt][:, h * VW:(h + 1) * VW],
                            start=(kt == 0), stop=(kt == NST - 1),
                        )
                rec = ptmp.tile([128, NSC], f32, tag="rec")
                nc.vector.reciprocal(out=rec[:], in_=avp[:, :, HD:HD + 1])
                for qt in range(4):
                    nc.vector.tensor_scalar_mul(
                        out=av_q[:, qt, hh], in0=avp[:, qt, 0:HD],
                        scalar1=rec[:, qt:qt + 1],
                    )

            def avt_transpose(qc, cs, av_q):
                # [q, (hh d)] -> [c, q] via the SBUF crossbar (bf16 2-byte)
                for qt in range(4):
                    qo = qc * SC + qt * 128
                    nc.sync.dma_start(
                        avt_sb[:, cs, qo:qo + 128],
                        av_q[:, qt].rearrange("p a b -> p (a b)"),
                        transpose=True,
                    )

            def o_proj(qc):
                for sti in range(4):
                    st = qc * 4 + sti
                    o_sb = pc.tile([128, 2, SC], f32, tag="o_sb")
                    for oc in range(2):
                        po = pop_pool.tile([128, SC], f32, tag="bp")
                        for cs in range(2):
                            nc.tensor.matmul(
                                po[:],
                                avt_sb[:, cs, st * 128:(st + 1) * 128],
                                wo_sb[:, cs, oc * SC:(oc + 1) * SC],
                                start=(cs == 0), stop=(cs == 1),
                            )
                        nc.vector.tensor_copy(out=o_sb[:, oc], in_=po[:])
                    nc.gpsimd.dma_start(
                        out[st * 128:(st + 1) * 128, :],
                        o_sb[:].rearrange("p a s -> p (a s)"))

            # k for all chunks (reshuffled once at full S), then q chunk 0:
            # the minimum for the first exp. Remaining q chunks and v tiles
            # interleave into the attention steps under the ACT exp budget.
            with tc.tile_pool(name="pa", bufs=3, space="PSUM") as pa_pool:
                qk_chunk(wk_sb, k_cs, 0, pa_pool)
                qk_chunk(wq_sb, q_cs, 0, pa_pool)
                for c in range(1, NSC):
                    qk_chunk(wk_sb, k_cs, c, pa_pool)

            pb_stack = tc.tile_pool(name="mm", bufs=2, space="PSUM")
            mm_pool = pb_stack.__enter__()
            pv_stack = tc.tile_pool(name="pvp", bufs=2, space="PSUM")
            pv_pool = pv_stack.__enter__()
            pop_stack = tc.tile_pool(name="pop", bufs=2, space="PSUM")
            pop_pool = pop_stack.__enter__()

            steps = [(qc, h) for qc in range(NSC) for h in range(HPC)]
            extras = {}
            vt = 0
            for i in range(4):
                for _ in range(4):
                    extras.setdefault(i, []).append(("v", vt)); vt += 1
            for i in range(3):
                extras.setdefault(i + 3, []).append(("q", i + 1))

            # software pipeline with AV lagging scores by 3 steps: the first
            # AV is emitted after every v tile (the scheduler only orders a
            # consumer against producers already emitted).
            # lag 3 while v/q interleaves finish, then catch up to kill
            # the pipeline tail
            lag_at = lambda i: 3 if i < 8 else (2 if i < 12 else 1)
            hist = {}
            drained = [0]
            av_q = [None]

            def drain(j):
                (pqc, ph), pp = hist.pop(j)
                if ph % 2 == 0:
                    av_q[0] = avq_pool.tile(
                        [128, NSC, 2, HD], bf16, tag="avq", name="av_q")
                av_norm(pqc, ph, pp, av_q[0])
                if ph % 2 == 1:
                    avt_transpose(pqc, ph // 2, av_q[0])
                if ph == HPC - 1:
                    o_proj(pqc)

            for i, step in enumerate(steps):
                hist[i] = (step, scores_exp(*step))
                for kind, idx in extras.get(i, []):
                    if kind == "q":
                        qk_chunk(wq_sb, q_cs, idx, mm_pool)
                    else:
                        v_tile(idx)
                while drained[0] <= i - lag_at(i):
                    drain(drained[0]); drained[0] += 1
            while drained[0] < len(steps):
                drain(drained[0]); drained[0] += 1
            pop_stack.__exit__(None, None, None)
            pv_stack.__exit__(None, None, None)
            pb_stack.__exit__(None, None, None)

    _split_waits(nc, mybir)
    return nc


def _rope_tables():
    import ml_dtypes
    inv_freq = 1.0 / (BASE ** (np.arange(0, HD, 2, dtype=np.float32) / HD))
    t = np.arange(S, dtype=np.float32)
    freqs = np.einsum("i,j->ij", t, inv_freq)        # [S, 32]
    cos_t = np.cos(freqs).T.astype(np.float32)       # [32, S]
    sin_t = np.sin(freqs).T.astype(np.float32)
    rows = np.arange(128) % 32
    cos2 = np.stack([cos_t[rows], cos_t[rows]], axis=1)      # [128, 2, S]
    sin2 = np.stack([-sin_t[rows], sin_t[rows]], axis=1)
    return (cos2.astype(ml_dtypes.bfloat16), sin2.astype(ml_dtypes.bfloat16))


def _run(inputs, trace=False):
    global _cached
    import ml_dtypes
    from concourse.bass_utils import run_bass_kernel_spmd

    x = np.asarray(inputs["x"], dtype=np.float32)
    wq = np.asarray(inputs["wq"], dtype=np.float32)
    wk = np.asarray(inputs["wk"], dtype=np.float32)
    wv = np.asarray(inputs["wv"], dtype=np.float32)
    wo = np.asarray(inputs["wo"], dtype=np.float32)
    bq = np.asarray(inputs["bq"], dtype=np.float32)
    bk = np.asarray(inputs["bk"], dtype=np.float32)
    bv = np.asarray(inputs["bv"], dtype=np.float32)
    bo = np.asarray(inputs["bo"], dtype=np.float32)
    assert not (bq.any() or bk.any() or bv.any()), \
        "nonzero qkv biases not supported by this kernel build"

    if _cached is None:
        _cached = _build()
    nc = _cached

    cos2, sin2 = _rope_tables()
    # L/H channel order: position p -> head p//32, channel p%32 (+32 for H)
    p = np.arange(128)
    lorder = (p // 32) * HD + (p % 32)
    order = np.concatenate([lorder, lorder + 32])
    bf = ml_dtypes.bfloat16
    in_maps = []
    for core in range(N_CORES):
        b, g = divmod(core, GROUPS)
        cs = slice(g * CPC, (g + 1) * CPC)
        in_maps.append({
            "xT": np.ascontiguousarray(x[b].T).astype(bf),
            "wqT": np.ascontiguousarray(wq[cs][order].T).astype(bf),
            "wkT": np.ascontiguousarray(wk[cs][order].T).astype(bf),
            "wvT": np.ascontiguousarray(wv[cs].T).astype(bf),
            "woT": np.ascontiguousarray(wo[:, cs].T).astype(bf),
            "cos2": cos2,
            "sin2": sin2,
        })

    res = run_bass_kernel_spmd(
        nc, in_maps, core_ids=list(range(N_CORES)), trace=trace)

    outp = np.zeros((B, S, HID), dtype=np.float32)
    for core in range(N_CORES):
        b = core // GROUPS
        outp[b] += res.results[core]["out"]
    outp += bo
    return outp, res


def kernel(**inputs):
    outp, _ = _run(inputs, trace=False)
    return outp
